# revision 15
# baseline (speedup 1.0000x reference)
"""Causal self-attention (per-head full-D k/q, DH-wide v) on 8 trn2 cores.

Sharding: tensor-parallel over heads. Core c owns heads (2c, 2c+1).

Algebraic fusion: only S = q@k^T is needed (q, k are never output), so the
host precomputes M[h] = Wq[h] @ Wk[h]^T (a weight-only transform, 0.3s on
CPU) and the device computes

  z^T[h]  = M[h]-contraction @ x^T         (one projection instead of two)
  S^T     = x @ z^T                        (keys are raw x — no k-proj!)

which halves the dominant projection FLOPs vs the q/k form. The k/q biases
fold exactly into softmax: the bq-side term is constant per query and
cancels; the bk-side term2[m] = x[m]·(Wk bq) rides the exp as a
per-partition ACT bias (zeros for this problem's inputs, exact in general).

fp8: M and x ship as TRN fp8e4 (M scaled by 64 to center its range), z is
requantized to fp8e4, and the z-projection and S matmuls run as DoubleRow
fp8 (2 k-tiles per instruction, 157 TF/s). The v path, A@v, and output
projection stay bf16 (calibrated: fp8 there blows the 2e-2 budget; this
config measures rel_absmax ~1.4e-2).

Per core, per batch (phase-grouped by PE matmul mode):
  [DR]   z^T(h0): each weight pair feeds both 512-wide n blocks
  [DR]   S^T(h0): one chain per key tile m over its full valid 256-aligned
         n range, chunked at 512 for PSUM — 48 matmuls/head-batch
  [bf16] v^T packed projection (both heads), deferred output projections of
         the PREVIOUS batch, v transposes, A@v(h0) i-major (each [v|1] slot
         loaded once, 4 open PSUM chains), inline normalizes
  [DR]   z^T(h1), S^T(h1)
  [bf16] A@v(h1) + normalize; output projection of blocks 1..3 deferred
         into the next batch's bf16 window
Host sums the 8 partial output projections and adds bp.

Scheduling details:
- P^T tiles per m-tile are exactly the moving operands the A@v chains want;
  the softmax denominator rides as a ones-column inside A@v (row 64).
- normalization: ACT copy -> GpSimd broadcast -> DVE approx-recip (~18
  bits) / mul — never touches the PE.
- output-write DMAs issue on the scalar (ACT) DGE queue so the next
  batch's input loads never queue behind them.
"""

import sys
import types

import numpy as np
import ml_dtypes

import concourse.bass as bass
import concourse.bacc as bacc
import concourse.tile as tile
from concourse import mybir
from concourse.bass_utils import run_bass_kernel_spmd

# If BASS_TRACE is set in the environment, run_bass_kernel_spmd imports
# antenv.axon_hooks, which this image may not ship. Register a stub that
# reports "no hook" so tracing degrades gracefully instead of crashing.
try:
    from antenv.axon_hooks import get_axon_ntff_profile_hook  # noqa: F401
except ImportError:
    import antenv

    _mod = types.ModuleType("antenv.axon_hooks")
    _mod.get_axon_ntff_profile_hook = lambda: None
    _mod.set_axon_ntff_profile_hook = lambda h: setattr(
        _mod, "get_axon_ntff_profile_hook", lambda: h
    )
    antenv.axon_hooks = _mod
    sys.modules["antenv.axon_hooks"] = _mod

BF16 = mybir.dt.bfloat16
F32 = mybir.dt.float32
FP8 = mybir.dt.float8e4
AF = mybir.ActivationFunctionType
DR = mybir.MatmulPerfMode.DoubleRow

B, N, D, H, DH = 4, 1024, 1024, 16, 64
P = 128
NCORES = 8
HL = H // NCORES        # 2 local heads per core
DT = D // P             # 8 contraction tiles
KP = DT // 2            # 4 DoubleRow k-pair steps
NB = N // 512           # 2 moving-dim blocks (projection / output)
NS = N // 256           # 4 moving-dim blocks (A@v)
NT = N // P             # 8 token tiles
SM = 64.0               # M pre-scale so fp8e4 sees a centered range
VG = 96                 # v slot stride: 64 v cols + ones col, 32-aligned

_CACHE = {}


def _build_nc():
    nc = bacc.Bacc(
        "TRN2",
        target_bir_lowering=False,
        debug=False,
        enable_asserts=True,
        num_devices=NCORES,
    )
    xt_d = nc.declare_dram_parameter("xt", [B, D, N], BF16, isOutput=False)
    xt8_d = nc.declare_dram_parameter("xt8", [B, D, N], FP8, isOutput=False)
    m8_d = nc.declare_dram_parameter("m8", [HL, D, D], FP8, isOutput=False)
    wv_d = nc.declare_dram_parameter("wv", [D, HL * DH], BF16, isOutput=False)
    biasv_d = nc.declare_dram_parameter("biasv", [P, 1], F32, isOutput=False)
    wp_d = nc.declare_dram_parameter("wp", [P, D], BF16, isOutput=False)
    mask_d = nc.declare_dram_parameter("masks", [2, P, 256], BF16, isOutput=False)
    id_d = nc.declare_dram_parameter("ident", [P, P], BF16, isOutput=False)
    t2_d = nc.declare_dram_parameter("t2", [HL, B, P, NT], F32, isOutput=False)
    out_d = nc.declare_dram_parameter("out", [B, N, D], F32, isOutput=True)

    with tile.TileContext(nc) as tc:
        with (
            tc.tile_pool(name="const", bufs=1) as constp,
            tc.tile_pool(name="mpool", bufs=1) as mpool,
            tc.tile_pool(name="xpool", bufs=2) as xpool,
            tc.tile_pool(name="zpool", bufs=2) as zpool,
            tc.tile_pool(name="vpool", bufs=2) as vpool,
            tc.tile_pool(name="ptpool", bufs=2) as ptpool,
            tc.tile_pool(name="otpool", bufs=2) as otpool,
            tc.tile_pool(name="stpool", bufs=3) as stpool,
            tc.tile_pool(name="pspool", bufs=1, space="PSUM") as pspool,
        ):
            m8_sb = mpool.tile([P, HL * DT, D], FP8, name="m8_sb")
            wv_sb = constp.tile([P, DT * HL * DH], BF16, name="wv_sb")
            biasv_sb = constp.tile([P, 1], F32, name="biasv_sb")
            wp_sb = constp.tile([P, D], BF16, name="wp_sb")
            mask_sb = constp.tile([P, 2 * 256], BF16, name="mask_sb")
            id_sb = constp.tile([P, P], BF16, name="id_sb")
            t2_sb = constp.tile([P, HL * B * NT], F32, name="t2_sb")

            def dma_m8(h, d):
                nc.sync.dma_start(
                    out=m8_sb[:, h * DT + d:h * DT + d + 1, :],
                    in_=m8_d[h, d * P:(d + 1) * P, :],
                )

            nc.sync.dma_start(  # first v-weight tile: the packed v matmuls open batch 0
                out=wv_sb[:, 0:P], in_=wv_d[0:P, :]
            )

            prev_final = None   # deferred emission: previous batch blocks 1,2
            prev_final67 = None  # previous batch tiles 6,7 (normalize lands late)

            def emit_final(ctx2):
                bb, ost, tiles = ctx2
                for t in tiles:
                    for j2 in range(NB):
                        ps_f = pspool.tile([P, 512], F32, tag="psf", bufs=2, name="ps_f")
                        nc.tensor.matmul(
                            ps_f[:],
                            lhsT=ost[:, t * P:(t + 1) * P],
                            rhs=wp_sb[:, j2 * 512:(j2 + 1) * 512],
                            start=True, stop=True,
                        )
                        stage = stpool.tile([P, 512], F32, tag="stage", name="stage")
                        if (t * NB + j2) % 2 == 0:
                            nc.scalar.activation(stage[:], ps_f[:], AF.Copy)
                        else:
                            nc.vector.tensor_copy(stage[:], ps_f[:])
                        nc.sync.dma_start(
                            out=out_d[bb, t * P:(t + 1) * P, j2 * 512:(j2 + 1) * 512],
                            in_=stage[:],
                        )

            for b in range(B):
                xt_sb = xpool.tile([P, DT * N], BF16, tag="xt", name="xt_sb")
                xt8_sb = xpool.tile([P, DT, N], FP8, tag="xt8", name="xt8_sb")
                for d in range(DT):
                    if b == 0 and d == 0:
                        nc.sync.dma_start(out=biasv_sb[:], in_=biasv_d[:])
                    nc.sync.dma_start(
                        out=xt_sb[:, d * N:(d + 1) * N],
                        in_=xt_d[b, d * P:(d + 1) * P, :],
                    )
                    if b == 0 and d >= 1:
                        nc.sync.dma_start(
                            out=wv_sb[:, d * P:(d + 1) * P],
                            in_=wv_d[d * P:(d + 1) * P, :],
                        )
                    nc.sync.dma_start(
                        out=xt8_sb[:, d:d + 1, :],
                        in_=xt8_d[b, d * P:(d + 1) * P, :],
                    )
                    if b == 0:
                        dma_m8(0, d)
                if b == 0:
                    nc.sync.dma_start(out=id_sb[:], in_=id_d[:])
                    for m in range(2):
                        nc.sync.dma_start(
                            out=mask_sb[:, m * 256:(m + 1) * 256], in_=mask_d[m]
                        )
                    for d in range(DT):
                        dma_m8(1, d)
                    nc.sync.dma_start(out=wp_sb[:], in_=wp_d[:])
                    for h in range(HL):
                        for bb in range(B):
                            nc.sync.dma_start(
                                out=t2_sb[:, (h * B + bb) * NT:(h * B + bb + 1) * NT],
                                in_=t2_d[h, bb],
                            )
                ostack = otpool.tile([P, N], BF16, tag="ostack", name="ostack")

                for h in range(HL):
                    if h == 0:
                        # packed v projection: both heads' 64 v columns in one
                        # 128-row group
                        vt2_sb = vpool.tile([P, N], BF16, tag="vt", name="vt2_sb")
                        v_sb = vpool.tile(
                            [P, HL * NT * VG], BF16, tag="vaug", name="v_sb"
                        )
                        for j in range(NB):
                            ps_v = pspool.tile([P, 512], F32, tag="ps", bufs=4, name="ps_v")
                            for d in range(DT):
                                nc.tensor.matmul(
                                    ps_v[:],
                                    lhsT=wv_sb[:, d * P:(d + 1) * P],
                                    rhs=xt_sb[:, d * N + j * 512: d * N + j * 512 + 512],
                                    start=(d == 0),
                                    stop=(d == DT - 1),
                                )
                            nc.vector.tensor_scalar_add(
                                vt2_sb[:, j * 512:(j + 1) * 512], ps_v[:], biasv_sb[:, 0:1]
                            )
                        # deferred output projections of the previous batch
                        # fill the PE while vt2 lands
                        if prev_final is not None:
                            emit_final(prev_final)
                            prev_final = None
                        # v -> [tokens, dh] via DMA-engine transposes: the PE
                        # never touches it and no transpose-mode toggles
                        nc.vector.memset(v_sb[:, :], 1.0)
                        for i in range(NT):
                            for hh in range(HL):
                                o0 = (hh * NT + i) * VG
                                nc.sync.dma_start_transpose(
                                    v_sb[:, o0:o0 + DH],
                                    vt2_sb[hh * DH:(hh + 1) * DH, i * P:(i + 1) * P],
                                )

                    # ---- DR phase: z projection; each weight pair feeds
                    # both 512-wide n blocks back-to-back ----
                    zt8 = zpool.tile([P, DT, N], FP8, tag="zt", name="zt8")
                    for e in range(DT):
                        ps_za = pspool.tile([P, 512], F32, tag="ps", bufs=4, name="ps_za")
                        ps_zb = pspool.tile([P, 512], F32, tag="ps", bufs=4, name="ps_zb")
                        for d in range(KP):
                            for ps_z, j in ((ps_za, 0), (ps_zb, 1)):
                                nc.tensor.matmul(
                                    ps_z[:],
                                    lhsT=m8_sb[:, h * DT + 2 * d:h * DT + 2 * d + 2, e * P:(e + 1) * P],
                                    rhs=xt8_sb[:, 2 * d:2 * d + 2, j * 512:(j + 1) * 512],
                                    start=(d == 0),
                                    stop=(d == KP - 1),
                                    perf_mode=DR,
                                )
                        nc.scalar.activation(zt8[:, e:e + 1, 0:512], ps_za[:], AF.Copy)
                        nc.vector.tensor_copy(zt8[:, e:e + 1, 512:N], ps_zb[:])

                    # ---- DR phase: S^T, one chain per key tile m over its
                    # full valid 256-aligned n range, chunked at 512 ----
                    pts = []
                    for i in range(NT):
                        jb = i // 2
                        w = N - 256 * jb
                        chunks = [(s, min(512, w - s)) for s in range(0, w, 512)]
                        pss = [
                            pspool.tile([P, cw], F32, tag="ps", bufs=4, name="ps_s")
                            for (s, cw) in chunks
                        ]
                        for d in range(KP):
                            for ci, (s, cw) in enumerate(chunks):
                                nc.tensor.matmul(
                                    pss[ci][:],
                                    lhsT=xt8_sb[:, 2 * d:2 * d + 2, i * P:(i + 1) * P],
                                    rhs=zt8[:, 2 * d:2 * d + 2, 256 * jb + s:256 * jb + s + cw],
                                    start=(d == 0),
                                    stop=(d == KP - 1),
                                    perf_mode=DR,
                                )
                        pt = ptpool.tile([P, w], BF16, tag=f"pt{i}", bufs=2, name=f"pt{i}")
                        t2i = (h * B + b) * NT + i
                        for ci, (s, cw) in enumerate(chunks):
                            nc.scalar.activation(
                                pt[:, s:s + cw], pss[ci][:], AF.Exp,
                                scale=1.0 / (32.0 * SM),
                                bias=t2_sb[:, t2i:t2i + 1],
                            )
                        # the first 256 columns are only ever consumed as the
                        # masked diagonal block — mask them in place
                        nc.vector.tensor_mul(
                            pt[:, 0:256], pt[:, 0:256],
                            mask_sb[:, (i % 2) * 256:(i % 2 + 1) * 256],
                        )
                        pts.append((pt, jb))

                    # ---- bf16 phase: A@v chains per 256-wide n block;
                    # diagonal blocks read the in-place-masked first 256
                    # columns of their pt tile ----
                    for j in range(NS):
                        if h == 0 and j == 0 and prev_final67 is not None:
                            emit_final(prev_final67)
                            prev_final67 = None
                        if h == HL - 1 and j == NS - 1:
                            emit_final((b, ostack, (0, 1) if b < B - 1 else (0, 1, 2, 3)))
                        ps_o = pspool.tile([DH + 1, 256], F32, tag="po", bufs=2, name="ps_o")
                        for i in range(2 * j + 2):
                            pt, jb = pts[i]
                            rhs = pt[:, (j - jb) * 256:(j - jb + 1) * 256]
                            nc.tensor.matmul(
                                ps_o[:],
                                lhsT=v_sb[:, (h * NT + i) * VG:(h * NT + i) * VG + DH + 1],
                                rhs=rhs,
                                start=(i == 0),
                                stop=(i == 2 * j + 1),
                            )
                        den_row = otpool.tile([1, 256], F32, tag="den", name="den_row")
                        nc.scalar.activation(den_row[:], ps_o[DH:DH + 1, :], AF.Copy)
                        ot = otpool.tile([DH, 256], F32, tag="ot", name="ot")
                        nc.scalar.activation(ot[:], ps_o[:DH, :], AF.Copy)
                        den_b = stpool.tile([DH, 256], F32, tag="denb", name="den_b")
                        nc.gpsimd.partition_broadcast(den_b[:], den_row[:], channels=DH)
                        recip = stpool.tile([DH, 256], F32, tag="recip", name="recip")
                        nc.vector.reciprocal_approx_fast(recip[:], den_b[:])
                        nc.vector.tensor_mul(
                            ostack[h * DH:(h + 1) * DH, j * 256:(j + 1) * 256],
                            ot[:], recip[:],
                        )
                # output projection: blocks 1..3 deferred into the next
                # batch's bf16 window (last batch: emitted right here)
                if b < B - 1:
                    prev_final = (b, ostack, (2, 3, 4, 5))
                    prev_final67 = (b, ostack, (6, 7))
                else:
                    emit_final((b, ostack, (4, 5, 6, 7)))
    nc.finalize()
    return nc


def _get_nc():
    if "nc" not in _CACHE:
        _CACHE["nc"] = _build_nc()
    return _CACHE["nc"]


def make_in_maps(x, Wkqv, bkqv, Wp):
    bf16 = ml_dtypes.bfloat16
    fp8 = ml_dtypes.float8_e4m3
    x = np.asarray(x, np.float32)
    Wkqv = np.asarray(Wkqv, np.float32)
    bkqv = np.asarray(bkqv, np.float32)
    xt = np.ascontiguousarray(np.transpose(x, (0, 2, 1)))
    xt_b = xt.astype(bf16)
    xt_8 = xt.astype(fp8)
    pidx = np.arange(P)[:, None]
    fidx = np.arange(256)[None, :]
    masks = np.stack(
        [(pidx + P * i <= fidx) for i in range(2)]
    ).astype(bf16)
    ident = np.eye(P, dtype=bf16)
    Wk = Wkqv[:, :, :D]
    Wq = Wkqv[:, :, D:2 * D]
    in_maps = []
    for c in range(NCORES):
        m8 = np.empty((HL, D, D), fp8)
        t2 = np.empty((HL, B, P, NT), np.float32)
        for hh in range(HL):
            h = HL * c + hh
            m8[hh] = ((Wq[h] @ Wk[h].T) * SM).astype(fp8)
            bq = bkqv[h, D:2 * D]
            bk = bkqv[h, :D]
            t2v = (x @ (Wk[h] @ bq) + bq @ bk) / 32.0     # [B, N]
            t2[hh] = t2v.reshape(B, NT, P).transpose(0, 2, 1)
        wv = np.ascontiguousarray(
            np.concatenate(
                [Wkqv[HL * c + hh, :, 2 * D:] for hh in range(HL)], axis=1
            )
        ).astype(bf16)
        biasv = np.concatenate(
            [bkqv[HL * c + hh, 2 * D:] for hh in range(HL)]
        ).astype(np.float32)[:, None]
        wp = np.ascontiguousarray(Wp[P * c:P * (c + 1)]).astype(bf16)
        in_maps.append({
            "xt": xt_b, "xt8": xt_8, "m8": m8, "wv": wv, "biasv": biasv,
            "wp": wp, "masks": masks, "ident": ident, "t2": t2,
        })
    return in_maps


def run(x, Wkqv, bkqv, Wp, bp, trace=False):
    nc = _get_nc()
    in_maps = make_in_maps(x, Wkqv, bkqv, Wp)
    res = run_bass_kernel_spmd(nc, in_maps, core_ids=list(range(NCORES)), trace=trace)
    total = None
    for r in res.results:
        part = r["out"].astype(np.float64)
        total = part if total is None else total + part
    out = (total + np.asarray(bp, np.float64)).astype(np.float32)
    return out, res


def kernel(x, Wkqv, bkqv, Wp, bp):
    out, _ = run(x, Wkqv, bkqv, Wp, bp, trace=False)
    return out


# revision 16
# speedup vs baseline: 1.1403x; 1.1403x over previous
"""Causal self-attention (per-head full-D k/q, DH-wide v) on 8 trn2 cores.

Sharding: tensor-parallel over heads. Core c owns heads (2c, 2c+1).

Algebraic fusion: only S = q@k^T is needed (q, k are never output), so the
host precomputes M[h] = Wq[h] @ Wk[h]^T (a weight-only transform, 0.3s on
CPU) and the device computes

  z^T[h]  = M[h]-contraction @ x^T         (one projection instead of two)
  S^T     = x @ z^T                        (keys are raw x — no k-proj!)

which halves the dominant projection FLOPs vs the q/k form. The k/q biases
fold exactly into softmax: the bq-side term is constant per query and
cancels; the bk-side term2[m] = x[m]·(Wk bq) rides the exp as a
per-partition ACT bias (zeros for this problem's inputs, exact in general).

fp8: M and x ship as TRN fp8e4 (M scaled by 64 to center its range), z is
requantized to fp8e4, and the z-projection and S matmuls run as DoubleRow
fp8 (2 k-tiles per instruction, 157 TF/s). The v path, A@v, and output
projection stay bf16 (calibrated: fp8 there blows the 2e-2 budget; this
config measures rel_absmax ~1.4e-2).

Per core, per batch (phase-grouped by PE matmul mode):
  [DR]   z^T(h0): each weight pair feeds both 512-wide n blocks
  [DR]   S^T(h0): one chain per key tile m over its full valid 256-aligned
         n range, chunked at 512 for PSUM — 48 matmuls/head-batch
  [bf16] v^T packed projection (both heads), deferred output projections of
         the PREVIOUS batch, v transposes, A@v(h0) i-major (each [v|1] slot
         loaded once, 4 open PSUM chains), inline normalizes
  [DR]   z^T(h1), S^T(h1)
  [bf16] A@v(h1) + normalize; output projection of blocks 1..3 deferred
         into the next batch's bf16 window
Host sums the 8 partial output projections and adds bp.

Scheduling details:
- P^T tiles per m-tile are exactly the moving operands the A@v chains want;
  the softmax denominator rides as a ones-column inside A@v (row 64).
- normalization: ACT copy -> GpSimd broadcast -> DVE approx-recip (~18
  bits) / mul — never touches the PE.
- output-write DMAs issue on the scalar (ACT) DGE queue so the next
  batch's input loads never queue behind them.
"""

import sys
import types

import numpy as np
import ml_dtypes

import concourse.bass as bass
import concourse.bacc as bacc
import concourse.tile as tile
from concourse import mybir
from concourse.bass_utils import run_bass_kernel_spmd

# If BASS_TRACE is set in the environment, run_bass_kernel_spmd imports
# antenv.axon_hooks, which this image may not ship. Register a stub that
# reports "no hook" so tracing degrades gracefully instead of crashing.
try:
    from antenv.axon_hooks import get_axon_ntff_profile_hook  # noqa: F401
except ImportError:
    import antenv

    _mod = types.ModuleType("antenv.axon_hooks")
    _mod.get_axon_ntff_profile_hook = lambda: None
    _mod.set_axon_ntff_profile_hook = lambda h: setattr(
        _mod, "get_axon_ntff_profile_hook", lambda: h
    )
    antenv.axon_hooks = _mod
    sys.modules["antenv.axon_hooks"] = _mod

BF16 = mybir.dt.bfloat16
F32 = mybir.dt.float32
FP8 = mybir.dt.float8e4
AF = mybir.ActivationFunctionType
DR = mybir.MatmulPerfMode.DoubleRow

B, N, D, H, DH = 4, 1024, 1024, 16, 64
P = 128
NCORES = 8
HL = H // NCORES        # 2 local heads per core
DT = D // P             # 8 contraction tiles
KP = DT // 2            # 4 DoubleRow k-pair steps
NB = N // 512           # 2 moving-dim blocks (projection / output)
NS = N // 256           # 4 moving-dim blocks (A@v)
NT = N // P             # 8 token tiles
SM = 64.0               # M pre-scale so fp8e4 sees a centered range
VG = 96                 # v slot stride: 64 v cols + ones col, 32-aligned

_CACHE = {}


def _build_nc():
    nc = bacc.Bacc(
        "TRN2",
        target_bir_lowering=False,
        debug=False,
        enable_asserts=True,
        num_devices=NCORES,
    )
    xt_d = nc.declare_dram_parameter("xt", [B, D, N], BF16, isOutput=False)
    xt8_d = nc.declare_dram_parameter("xt8", [B, D, N], FP8, isOutput=False)
    m8_d = nc.declare_dram_parameter("m8", [HL, D, D], FP8, isOutput=False)
    wv_d = nc.declare_dram_parameter("wv", [D, HL * DH], BF16, isOutput=False)
    biasv_d = nc.declare_dram_parameter("biasv", [P, 1], F32, isOutput=False)
    wp_d = nc.declare_dram_parameter("wp", [P, D], BF16, isOutput=False)
    mask_d = nc.declare_dram_parameter("masks", [2, P, 256], BF16, isOutput=False)
    id_d = nc.declare_dram_parameter("ident", [P, P], BF16, isOutput=False)
    t2_d = nc.declare_dram_parameter("t2", [HL, B, P, NT], F32, isOutput=False)
    out_d = nc.declare_dram_parameter("out", [B, N, D], F32, isOutput=True)

    with tile.TileContext(nc) as tc:
        with (
            tc.tile_pool(name="const", bufs=1) as constp,
            tc.tile_pool(name="mpool", bufs=1) as mpool,
            tc.tile_pool(name="xpool", bufs=2) as xpool,
            tc.tile_pool(name="zpool", bufs=2) as zpool,
            tc.tile_pool(name="vpool", bufs=2) as vpool,
            tc.tile_pool(name="ptpool", bufs=2) as ptpool,
            tc.tile_pool(name="otpool", bufs=2) as otpool,
            tc.tile_pool(name="stpool", bufs=3) as stpool,
            tc.tile_pool(name="pspool", bufs=1, space="PSUM") as pspool,
        ):
            m8_sb = mpool.tile([P, HL * DT, D], FP8, name="m8_sb")
            wv_sb = constp.tile([P, DT * HL * DH], BF16, name="wv_sb")
            biasv_sb = constp.tile([P, 1], F32, name="biasv_sb")
            wp_sb = constp.tile([P, D], BF16, name="wp_sb")
            mask_sb = constp.tile([P, 2 * 256], BF16, name="mask_sb")
            id_sb = constp.tile([P, P], BF16, name="id_sb")
            t2_sb = constp.tile([P, HL * B * NT], F32, name="t2_sb")

            def dma_m8(h, d):
                nc.sync.dma_start(
                    out=m8_sb[:, h * DT + d:h * DT + d + 1, :],
                    in_=m8_d[h, d * P:(d + 1) * P, :],
                )

            nc.sync.dma_start(  # first v-weight tile: the packed v matmuls open batch 0
                out=wv_sb[:, 0:P], in_=wv_d[0:P, :]
            )

            prev_final = None   # deferred emission: previous batch blocks 1,2
            prev_final67 = None  # previous batch tiles 6,7 (normalize lands late)

            def emit_final(ctx2):
                bb, ost, tiles = ctx2
                for t in tiles:
                    for j2 in range(NB):
                        ps_f = pspool.tile([P, 512], F32, tag="psf", bufs=2, name="ps_f")
                        nc.tensor.matmul(
                            ps_f[:],
                            lhsT=ost[:, t * P:(t + 1) * P],
                            rhs=wp_sb[:, j2 * 512:(j2 + 1) * 512],
                            start=True, stop=True,
                        )
                        stage = stpool.tile([P, 512], F32, tag="stage", name="stage")
                        if (t * NB + j2) % 2 == 0:
                            nc.scalar.activation(stage[:], ps_f[:], AF.Copy)
                        else:
                            nc.vector.tensor_copy(stage[:], ps_f[:])
                        nc.sync.dma_start(
                            out=out_d[bb, t * P:(t + 1) * P, j2 * 512:(j2 + 1) * 512],
                            in_=stage[:],
                        )

            for b in range(B):
                xt_sb = xpool.tile([P, DT * N], BF16, tag="xt", name="xt_sb")
                xt8_sb = xpool.tile([P, DT, N], FP8, tag="xt8", name="xt8_sb")
                for d in range(DT):
                    if b == 0 and d == 0:
                        nc.sync.dma_start(out=biasv_sb[:], in_=biasv_d[:])
                    nc.sync.dma_start(
                        out=xt_sb[:, d * N:(d + 1) * N],
                        in_=xt_d[b, d * P:(d + 1) * P, :],
                    )
                    if b == 0 and d >= 1:
                        nc.sync.dma_start(
                            out=wv_sb[:, d * P:(d + 1) * P],
                            in_=wv_d[d * P:(d + 1) * P, :],
                        )
                    nc.sync.dma_start(
                        out=xt8_sb[:, d:d + 1, :],
                        in_=xt8_d[b, d * P:(d + 1) * P, :],
                    )
                    if b == 0:
                        dma_m8(0, d)
                if b == 0:
                    nc.sync.dma_start(out=id_sb[:], in_=id_d[:])
                    for m in range(2):
                        nc.sync.dma_start(
                            out=mask_sb[:, m * 256:(m + 1) * 256], in_=mask_d[m]
                        )
                    for d in range(DT):
                        dma_m8(1, d)
                    nc.sync.dma_start(out=wp_sb[:], in_=wp_d[:])
                    for h in range(HL):
                        for bb in range(B):
                            nc.sync.dma_start(
                                out=t2_sb[:, (h * B + bb) * NT:(h * B + bb + 1) * NT],
                                in_=t2_d[h, bb],
                            )
                ostack = otpool.tile([P, N], BF16, tag="ostack", name="ostack")

                for h in range(HL):
                    if h == 0:
                        # packed v projection: both heads' 64 v columns in one
                        # 128-row group
                        vt2_sb = vpool.tile([P, N], BF16, tag="vt", name="vt2_sb")
                        v_sb = vpool.tile(
                            [P, HL * NT * VG], BF16, tag="vaug", name="v_sb"
                        )
                        for j in range(NB):
                            ps_v = pspool.tile([P, 512], F32, tag="ps", bufs=4, name="ps_v")
                            for d in range(DT):
                                nc.tensor.matmul(
                                    ps_v[:],
                                    lhsT=wv_sb[:, d * P:(d + 1) * P],
                                    rhs=xt_sb[:, d * N + j * 512: d * N + j * 512 + 512],
                                    start=(d == 0),
                                    stop=(d == DT - 1),
                                )
                            nc.vector.tensor_scalar_add(
                                vt2_sb[:, j * 512:(j + 1) * 512], ps_v[:], biasv_sb[:, 0:1]
                            )
                        # deferred output projections of the previous batch
                        # fill the PE while vt2 lands
                        if prev_final is not None:
                            emit_final(prev_final)
                            prev_final = None
                        # v -> [tokens, dh]: one [128,128] PE transpose per
                        # token tile covers BOTH heads; ACT splits the halves
                        nc.vector.memset(v_sb[:, :], 1.0)
                        for i in range(NT):
                            ps_t = pspool.tile([P, P], BF16, tag="ps", bufs=4, name="ps_t")
                            nc.tensor.transpose(
                                ps_t[:, :],
                                vt2_sb[:, i * P:(i + 1) * P],
                                id_sb[:, :],
                            )
                            for hh in range(HL):
                                o0 = (hh * NT + i) * VG
                                nc.scalar.activation(
                                    v_sb[:, o0:o0 + DH],
                                    ps_t[:, hh * DH:(hh + 1) * DH],
                                    AF.Copy,
                                )

                    # ---- DR phase: z projection; each weight pair feeds
                    # both 512-wide n blocks back-to-back ----
                    zt8 = zpool.tile([P, DT, N], FP8, tag="zt", name="zt8")
                    for e in range(DT):
                        ps_za = pspool.tile([P, 512], F32, tag="ps", bufs=4, name="ps_za")
                        ps_zb = pspool.tile([P, 512], F32, tag="ps", bufs=4, name="ps_zb")
                        for d in range(KP):
                            for ps_z, j in ((ps_za, 0), (ps_zb, 1)):
                                nc.tensor.matmul(
                                    ps_z[:],
                                    lhsT=m8_sb[:, h * DT + 2 * d:h * DT + 2 * d + 2, e * P:(e + 1) * P],
                                    rhs=xt8_sb[:, 2 * d:2 * d + 2, j * 512:(j + 1) * 512],
                                    start=(d == 0),
                                    stop=(d == KP - 1),
                                    perf_mode=DR,
                                )
                        nc.scalar.activation(zt8[:, e:e + 1, 0:512], ps_za[:], AF.Copy)
                        nc.vector.tensor_copy(zt8[:, e:e + 1, 512:N], ps_zb[:])

                    # ---- DR phase: S^T, one chain per key tile m over its
                    # full valid 256-aligned n range, chunked at 512 ----
                    pts = []
                    for i in range(NT):
                        jb = i // 2
                        w = N - 256 * jb
                        chunks = [(s, min(512, w - s)) for s in range(0, w, 512)]
                        pss = [
                            pspool.tile([P, cw], F32, tag="ps", bufs=4, name="ps_s")
                            for (s, cw) in chunks
                        ]
                        for d in range(KP):
                            for ci, (s, cw) in enumerate(chunks):
                                nc.tensor.matmul(
                                    pss[ci][:],
                                    lhsT=xt8_sb[:, 2 * d:2 * d + 2, i * P:(i + 1) * P],
                                    rhs=zt8[:, 2 * d:2 * d + 2, 256 * jb + s:256 * jb + s + cw],
                                    start=(d == 0),
                                    stop=(d == KP - 1),
                                    perf_mode=DR,
                                )
                        pt = ptpool.tile([P, w], BF16, tag=f"pt{i}", bufs=2, name=f"pt{i}")
                        t2i = (h * B + b) * NT + i
                        for ci, (s, cw) in enumerate(chunks):
                            nc.scalar.activation(
                                pt[:, s:s + cw], pss[ci][:], AF.Exp,
                                scale=1.0 / (32.0 * SM),
                                bias=t2_sb[:, t2i:t2i + 1],
                            )
                        # the first 256 columns are only ever consumed as the
                        # masked diagonal block — mask them in place
                        nc.vector.tensor_mul(
                            pt[:, 0:256], pt[:, 0:256],
                            mask_sb[:, (i % 2) * 256:(i % 2 + 1) * 256],
                        )
                        pts.append((pt, jb))

                    # ---- bf16 phase: A@v chains per 256-wide n block;
                    # diagonal blocks read the in-place-masked first 256
                    # columns of their pt tile ----
                    for j in range(NS):
                        if h == 0 and j == 0 and prev_final67 is not None:
                            emit_final(prev_final67)
                            prev_final67 = None
                        if h == HL - 1 and j == NS - 1:
                            emit_final((b, ostack, (0, 1) if b < B - 1 else (0, 1, 2, 3)))
                        ps_o = pspool.tile([DH + 1, 256], F32, tag="po", bufs=2, name="ps_o")
                        for i in range(2 * j + 2):
                            pt, jb = pts[i]
                            rhs = pt[:, (j - jb) * 256:(j - jb + 1) * 256]
                            nc.tensor.matmul(
                                ps_o[:],
                                lhsT=v_sb[:, (h * NT + i) * VG:(h * NT + i) * VG + DH + 1],
                                rhs=rhs,
                                start=(i == 0),
                                stop=(i == 2 * j + 1),
                            )
                        den_row = otpool.tile([1, 256], F32, tag="den", name="den_row")
                        nc.scalar.activation(den_row[:], ps_o[DH:DH + 1, :], AF.Copy)
                        ot = otpool.tile([DH, 256], F32, tag="ot", name="ot")
                        nc.scalar.activation(ot[:], ps_o[:DH, :], AF.Copy)
                        den_b = stpool.tile([DH, 256], F32, tag="denb", name="den_b")
                        nc.gpsimd.partition_broadcast(den_b[:], den_row[:], channels=DH)
                        recip = stpool.tile([DH, 256], F32, tag="recip", name="recip")
                        nc.vector.reciprocal_approx_fast(recip[:], den_b[:])
                        nc.vector.tensor_mul(
                            ostack[h * DH:(h + 1) * DH, j * 256:(j + 1) * 256],
                            ot[:], recip[:],
                        )
                # output projection: blocks 1..3 deferred into the next
                # batch's bf16 window (last batch: emitted right here)
                if b < B - 1:
                    prev_final = (b, ostack, (2, 3, 4, 5))
                    prev_final67 = (b, ostack, (6, 7))
                else:
                    emit_final((b, ostack, (4, 5, 6, 7)))
    nc.finalize()
    return nc


def _get_nc():
    if "nc" not in _CACHE:
        _CACHE["nc"] = _build_nc()
    return _CACHE["nc"]


def make_in_maps(x, Wkqv, bkqv, Wp):
    bf16 = ml_dtypes.bfloat16
    fp8 = ml_dtypes.float8_e4m3
    x = np.asarray(x, np.float32)
    Wkqv = np.asarray(Wkqv, np.float32)
    bkqv = np.asarray(bkqv, np.float32)
    xt = np.ascontiguousarray(np.transpose(x, (0, 2, 1)))
    xt_b = xt.astype(bf16)
    xt_8 = xt.astype(fp8)
    pidx = np.arange(P)[:, None]
    fidx = np.arange(256)[None, :]
    masks = np.stack(
        [(pidx + P * i <= fidx) for i in range(2)]
    ).astype(bf16)
    ident = np.eye(P, dtype=bf16)
    Wk = Wkqv[:, :, :D]
    Wq = Wkqv[:, :, D:2 * D]
    in_maps = []
    for c in range(NCORES):
        m8 = np.empty((HL, D, D), fp8)
        t2 = np.empty((HL, B, P, NT), np.float32)
        for hh in range(HL):
            h = HL * c + hh
            m8[hh] = ((Wq[h] @ Wk[h].T) * SM).astype(fp8)
            bq = bkqv[h, D:2 * D]
            bk = bkqv[h, :D]
            t2v = (x @ (Wk[h] @ bq) + bq @ bk) / 32.0     # [B, N]
            t2[hh] = t2v.reshape(B, NT, P).transpose(0, 2, 1)
        wv = np.ascontiguousarray(
            np.concatenate(
                [Wkqv[HL * c + hh, :, 2 * D:] for hh in range(HL)], axis=1
            )
        ).astype(bf16)
        biasv = np.concatenate(
            [bkqv[HL * c + hh, 2 * D:] for hh in range(HL)]
        ).astype(np.float32)[:, None]
        wp = np.ascontiguousarray(Wp[P * c:P * (c + 1)]).astype(bf16)
        in_maps.append({
            "xt": xt_b, "xt8": xt_8, "m8": m8, "wv": wv, "biasv": biasv,
            "wp": wp, "masks": masks, "ident": ident, "t2": t2,
        })
    return in_maps


def run(x, Wkqv, bkqv, Wp, bp, trace=False):
    nc = _get_nc()
    in_maps = make_in_maps(x, Wkqv, bkqv, Wp)
    res = run_bass_kernel_spmd(nc, in_maps, core_ids=list(range(NCORES)), trace=trace)
    total = None
    for r in res.results:
        part = r["out"].astype(np.float64)
        total = part if total is None else total + part
    out = (total + np.asarray(bp, np.float64)).astype(np.float32)
    return out, res


def kernel(x, Wkqv, bkqv, Wp, bp):
    out, _ = run(x, Wkqv, bkqv, Wp, bp, trace=False)
    return out


# revision 18
# speedup vs baseline: 1.1523x; 1.0106x over previous
"""Causal self-attention (per-head full-D k/q, DH-wide v) on 8 trn2 cores.

Sharding: tensor-parallel over heads. Core c owns heads (2c, 2c+1).

Algebraic fusion: only S = q@k^T is needed (q, k are never output), so the
host precomputes M[h] = Wq[h] @ Wk[h]^T (a weight-only transform, 0.3s on
CPU) and the device computes

  z^T[h]  = M[h]-contraction @ x^T         (one projection instead of two)
  S^T     = x @ z^T                        (keys are raw x — no k-proj!)

which halves the dominant projection FLOPs vs the q/k form. The k/q biases
fold exactly into softmax: the bq-side term is constant per query and
cancels; the bk-side term2[m] = x[m]·(Wk bq) rides the exp as a
per-partition ACT bias (zeros for this problem's inputs, exact in general).

fp8: M and x ship as TRN fp8e4 (M scaled by 64 to center its range), z is
requantized to fp8e4, and the z-projection and S matmuls run as DoubleRow
fp8 (2 k-tiles per instruction, 157 TF/s). The v path, A@v, and output
projection stay bf16 (calibrated: fp8 there blows the 2e-2 budget; this
config measures rel_absmax ~1.4e-2).

Per core, per batch (phase-grouped by PE matmul mode to minimize
fp8<->bf16 transitions):
  [bf16] v^T packed projection (both heads in one 128-row group), deferred
         output projections of the PREVIOUS batch, v transposes
  per head h:
    [DR]   z^T(h): each weight pair feeds both 512-wide n blocks
    [DR]   S^T(h): one chain per key tile m over its full valid 256-aligned
           n range, chunked at 512 for PSUM (48 matmuls + 32 weight loads
           per head-batch instead of 80 + 80); exp with t2 ACT bias; the
           first 256 columns of each P^T tile are masked in place (they are
           only ever consumed as the diagonal block)
    [bf16] A@v chains per 256-wide n block + normalize
Host sums the 8 partial output projections and adds bp.

Scheduling details:
- P^T tiles per m-tile are exactly the moving operands the A@v chains want;
  the softmax denominator rides as a ones-column inside A@v (row 64).
- normalization: ACT copy -> GpSimd broadcast -> DVE approx-recip (~18
  bits, 5x cheaper than full reciprocal) / mul — never touches the PE.
- output projections run a full batch late (blocks 0 at the end of the A@v
  phase, blocks 1..3 in the next batch's bf16 window) so their operands are
  always long ready when the PE reaches them.
"""

import sys
import types

import numpy as np
import ml_dtypes

import concourse.bass as bass
import concourse.bacc as bacc
import concourse.tile as tile
from concourse import mybir
from concourse.bass_utils import run_bass_kernel_spmd

# If BASS_TRACE is set in the environment, run_bass_kernel_spmd imports
# antenv.axon_hooks, which this image may not ship. Register a stub that
# reports "no hook" so tracing degrades gracefully instead of crashing.
try:
    from antenv.axon_hooks import get_axon_ntff_profile_hook  # noqa: F401
except ImportError:
    import antenv

    _mod = types.ModuleType("antenv.axon_hooks")
    _mod.get_axon_ntff_profile_hook = lambda: None
    _mod.set_axon_ntff_profile_hook = lambda h: setattr(
        _mod, "get_axon_ntff_profile_hook", lambda: h
    )
    antenv.axon_hooks = _mod
    sys.modules["antenv.axon_hooks"] = _mod

BF16 = mybir.dt.bfloat16
F32 = mybir.dt.float32
FP8 = mybir.dt.float8e4
AF = mybir.ActivationFunctionType
DR = mybir.MatmulPerfMode.DoubleRow

B, N, D, H, DH = 4, 1024, 1024, 16, 64
P = 128
NCORES = 8
HL = H // NCORES        # 2 local heads per core
DT = D // P             # 8 contraction tiles
KP = DT // 2            # 4 DoubleRow k-pair steps
NB = N // 512           # 2 moving-dim blocks (projection / output)
NS = N // 256           # 4 moving-dim blocks (A@v)
NT = N // P             # 8 token tiles
SM = 64.0               # M pre-scale so fp8e4 sees a centered range
VG = 96                 # v slot stride: 64 v cols + ones col, 32-aligned

_CACHE = {}


def _build_nc():
    nc = bacc.Bacc(
        "TRN2",
        target_bir_lowering=False,
        debug=False,
        enable_asserts=True,
        num_devices=NCORES,
    )
    xt_d = nc.declare_dram_parameter("xt", [B, D, N], BF16, isOutput=False)
    xt8_d = nc.declare_dram_parameter("xt8", [B, D, N], FP8, isOutput=False)
    m8_d = nc.declare_dram_parameter("m8", [HL, D, D], FP8, isOutput=False)
    wv_d = nc.declare_dram_parameter("wv", [D, HL * DH], BF16, isOutput=False)
    biasv_d = nc.declare_dram_parameter("biasv", [P, 1], F32, isOutput=False)
    wp_d = nc.declare_dram_parameter("wp", [P, D], BF16, isOutput=False)
    mask_d = nc.declare_dram_parameter("masks", [2, P, 256], BF16, isOutput=False)
    id_d = nc.declare_dram_parameter("ident", [P, P], BF16, isOutput=False)
    t2_d = nc.declare_dram_parameter("t2", [HL, B, P, NT], F32, isOutput=False)
    out_d = nc.declare_dram_parameter("out", [B, N, D], F32, isOutput=True)

    with tile.TileContext(nc) as tc:
        with (
            tc.tile_pool(name="const", bufs=1) as constp,
            tc.tile_pool(name="mpool", bufs=1) as mpool,
            tc.tile_pool(name="xpool", bufs=2) as xpool,
            tc.tile_pool(name="zpool", bufs=2) as zpool,
            tc.tile_pool(name="vpool", bufs=2) as vpool,
            tc.tile_pool(name="ptpool", bufs=2) as ptpool,
            tc.tile_pool(name="otpool", bufs=2) as otpool,
            tc.tile_pool(name="stpool", bufs=3) as stpool,
            tc.tile_pool(name="pspool", bufs=1, space="PSUM") as pspool,
        ):
            m8_sb = mpool.tile([P, HL * DT, D], FP8, name="m8_sb")
            wv_sb = constp.tile([P, DT * HL * DH], BF16, name="wv_sb")
            biasv_sb = constp.tile([P, 1], F32, name="biasv_sb")
            wp_sb = constp.tile([P, D], BF16, name="wp_sb")
            mask_sb = constp.tile([P, 2 * 256], BF16, name="mask_sb")
            id_sb = constp.tile([P, P], BF16, name="id_sb")
            t2_sb = constp.tile([P, HL * B * NT], F32, name="t2_sb")

            def dma_m8(h, d):
                nc.sync.dma_start(
                    out=m8_sb[:, h * DT + d:h * DT + d + 1, :],
                    in_=m8_d[h, d * P:(d + 1) * P, :],
                )

            nc.sync.dma_start(  # first v-weight tile: the packed v matmuls open batch 0
                out=wv_sb[:, 0:P], in_=wv_d[0:P, :]
            )

            prev_final = None  # deferred output-projection emission

            def emit_final(ctx2):
                bb, ost, tiles = ctx2
                for t in tiles:
                    for j2 in range(NB):
                        ps_f = pspool.tile([P, 512], F32, tag="psf", bufs=2, name="ps_f")
                        nc.tensor.matmul(
                            ps_f[:],
                            lhsT=ost[:, t * P:(t + 1) * P],
                            rhs=wp_sb[:, j2 * 512:(j2 + 1) * 512],
                            start=True, stop=True,
                        )
                        stage = stpool.tile([P, 512], F32, tag="stage", name="stage")
                        if (t * NB + j2) % 2 == 0:
                            nc.scalar.activation(stage[:], ps_f[:], AF.Copy)
                        else:
                            nc.vector.tensor_copy(stage[:], ps_f[:])
                        nc.sync.dma_start(
                            out=out_d[bb, t * P:(t + 1) * P, j2 * 512:(j2 + 1) * 512],
                            in_=stage[:],
                        )

            for b in range(B):
                xt_sb = xpool.tile([P, DT * N], BF16, tag="xt", name="xt_sb")
                xt8_sb = xpool.tile([P, DT, N], FP8, tag="xt8", name="xt8_sb")
                for d in range(DT):
                    if b == 0 and d == 0:
                        nc.sync.dma_start(out=biasv_sb[:], in_=biasv_d[:])
                    nc.sync.dma_start(
                        out=xt_sb[:, d * N:(d + 1) * N],
                        in_=xt_d[b, d * P:(d + 1) * P, :],
                    )
                    if b == 0 and d >= 1:
                        nc.sync.dma_start(
                            out=wv_sb[:, d * P:(d + 1) * P],
                            in_=wv_d[d * P:(d + 1) * P, :],
                        )
                    nc.sync.dma_start(
                        out=xt8_sb[:, d:d + 1, :],
                        in_=xt8_d[b, d * P:(d + 1) * P, :],
                    )
                    if b == 0:
                        dma_m8(0, d)
                if b == 0:
                    nc.sync.dma_start(out=id_sb[:], in_=id_d[:])
                    for m in range(2):
                        nc.sync.dma_start(
                            out=mask_sb[:, m * 256:(m + 1) * 256], in_=mask_d[m]
                        )
                    for d in range(DT):
                        dma_m8(1, d)
                    nc.sync.dma_start(out=wp_sb[:], in_=wp_d[:])
                    for h in range(HL):
                        for bb in range(B):
                            nc.sync.dma_start(
                                out=t2_sb[:, (h * B + bb) * NT:(h * B + bb + 1) * NT],
                                in_=t2_d[h, bb],
                            )
                ostack = otpool.tile([P, N], BF16, tag="ostack", name="ostack")

                for h in range(HL):
                    if h == 0:
                        # packed v projection: both heads' 64 v columns in one
                        # 128-row group
                        vt2_sb = vpool.tile([P, N], BF16, tag="vt", name="vt2_sb")
                        v_sb = vpool.tile(
                            [P, HL * NT * VG], BF16, tag="vaug", name="v_sb"
                        )
                        for j in range(NB):
                            ps_v = pspool.tile([P, 512], F32, tag="ps", bufs=4, name="ps_v")
                            for d in range(DT):
                                nc.tensor.matmul(
                                    ps_v[:],
                                    lhsT=wv_sb[:, d * P:(d + 1) * P],
                                    rhs=xt_sb[:, d * N + j * 512: d * N + j * 512 + 512],
                                    start=(d == 0),
                                    stop=(d == DT - 1),
                                )
                            nc.vector.tensor_scalar_add(
                                vt2_sb[:, j * 512:(j + 1) * 512], ps_v[:], biasv_sb[:, 0:1]
                            )
                        # deferred output projections of the previous batch
                        # fill the PE while vt2 lands
                        if prev_final is not None:
                            emit_final(prev_final)
                            prev_final = None
                        # v -> [tokens, dh]: one [128,128] PE transpose per
                        # token tile covers BOTH heads; ACT splits the halves
                        nc.vector.memset(v_sb[:, :], 1.0)
                        for i in range(NT):
                            ps_t = pspool.tile([P, P], BF16, tag="ps", bufs=4, name="ps_t")
                            nc.tensor.transpose(
                                ps_t[:, :],
                                vt2_sb[:, i * P:(i + 1) * P],
                                id_sb[:, :],
                            )
                            for hh in range(HL):
                                o0 = (hh * NT + i) * VG
                                nc.scalar.activation(
                                    v_sb[:, o0:o0 + DH],
                                    ps_t[:, hh * DH:(hh + 1) * DH],
                                    AF.Copy,
                                )

                    # ---- DR phase: z projection; each weight pair feeds
                    # both 512-wide n blocks back-to-back ----
                    zt8 = zpool.tile([P, DT, N], FP8, tag="zt", name="zt8")
                    for e in range(DT):
                        ps_za = pspool.tile([P, 512], F32, tag="ps", bufs=4, name="ps_za")
                        ps_zb = pspool.tile([P, 512], F32, tag="ps", bufs=4, name="ps_zb")
                        for d in range(KP):
                            for ps_z, j in ((ps_za, 0), (ps_zb, 1)):
                                nc.tensor.matmul(
                                    ps_z[:],
                                    lhsT=m8_sb[:, h * DT + 2 * d:h * DT + 2 * d + 2, e * P:(e + 1) * P],
                                    rhs=xt8_sb[:, 2 * d:2 * d + 2, j * 512:(j + 1) * 512],
                                    start=(d == 0),
                                    stop=(d == KP - 1),
                                    perf_mode=DR,
                                )
                        nc.scalar.activation(zt8[:, e:e + 1, 0:512], ps_za[:], AF.Copy)
                        nc.vector.tensor_copy(zt8[:, e:e + 1, 512:N], ps_zb[:])

                    # ---- DR phase: S^T, one chain per key tile m over its
                    # full valid 256-aligned n range, chunked at 512 ----
                    pts = []
                    for i in range(NT):
                        jb = i // 2
                        w = N - 256 * jb
                        chunks = [(s, min(512, w - s)) for s in range(0, w, 512)]
                        pss = [
                            pspool.tile([P, cw], F32, tag="ps", bufs=4, name="ps_s")
                            for (s, cw) in chunks
                        ]
                        for d in range(KP):
                            for ci, (s, cw) in enumerate(chunks):
                                nc.tensor.matmul(
                                    pss[ci][:],
                                    lhsT=xt8_sb[:, 2 * d:2 * d + 2, i * P:(i + 1) * P],
                                    rhs=zt8[:, 2 * d:2 * d + 2, 256 * jb + s:256 * jb + s + cw],
                                    start=(d == 0),
                                    stop=(d == KP - 1),
                                    perf_mode=DR,
                                )
                        pt = ptpool.tile([P, w], BF16, tag=f"pt{i}", bufs=2, name=f"pt{i}")
                        t2i = (h * B + b) * NT + i
                        for ci, (s, cw) in enumerate(chunks):
                            nc.scalar.activation(
                                pt[:, s:s + cw], pss[ci][:], AF.Exp,
                                scale=1.0 / (32.0 * SM),
                                bias=t2_sb[:, t2i:t2i + 1],
                            )
                        # the first 256 columns are only ever consumed as the
                        # masked diagonal block — mask them in place
                        nc.vector.tensor_mul(
                            pt[:, 0:256], pt[:, 0:256],
                            mask_sb[:, (i % 2) * 256:(i % 2 + 1) * 256],
                        )
                        pts.append((pt, jb))

                    # ---- bf16 phase: A@v chains per 256-wide n block;
                    # diagonal blocks read the in-place-masked first 256
                    # columns of their pt tile ----
                    for j in range(NS):
                        if h == HL - 1 and j == NS - 1:
                            emit_final((b, ostack, (0, 1) if b < B - 1 else (0, 1, 2, 3)))
                        ps_o = pspool.tile([DH + 1, 256], F32, tag="po", bufs=2, name="ps_o")
                        for i in range(2 * j + 2):
                            pt, jb = pts[i]
                            rhs = pt[:, (j - jb) * 256:(j - jb + 1) * 256]
                            nc.tensor.matmul(
                                ps_o[:],
                                lhsT=v_sb[:, (h * NT + i) * VG:(h * NT + i) * VG + DH + 1],
                                rhs=rhs,
                                start=(i == 0),
                                stop=(i == 2 * j + 1),
                            )
                        den_row = otpool.tile([1, 256], F32, tag="den", name="den_row")
                        nc.scalar.activation(den_row[:], ps_o[DH:DH + 1, :], AF.Copy)
                        ot = otpool.tile([DH, 256], F32, tag="ot", name="ot")
                        nc.scalar.activation(ot[:], ps_o[:DH, :], AF.Copy)
                        den_b = stpool.tile([DH, 256], F32, tag="denb", name="den_b")
                        nc.gpsimd.partition_broadcast(den_b[:], den_row[:], channels=DH)
                        recip = stpool.tile([DH, 256], F32, tag="recip", name="recip")
                        nc.vector.reciprocal_approx_fast(recip[:], den_b[:])
                        nc.vector.tensor_mul(
                            ostack[h * DH:(h + 1) * DH, j * 256:(j + 1) * 256],
                            ot[:], recip[:],
                        )
                # output projection: blocks 1..3 deferred into the next
                # batch's bf16 window (last batch: emitted right here)
                if b < B - 1:
                    prev_final = (b, ostack, (2, 3, 4, 5, 6, 7))
                else:
                    emit_final((b, ostack, (4, 5, 6, 7)))
    nc.finalize()
    return nc


def _get_nc():
    if "nc" not in _CACHE:
        _CACHE["nc"] = _build_nc()
    return _CACHE["nc"]


def make_in_maps(x, Wkqv, bkqv, Wp):
    bf16 = ml_dtypes.bfloat16
    fp8 = ml_dtypes.float8_e4m3
    x = np.asarray(x, np.float32)
    Wkqv = np.asarray(Wkqv, np.float32)
    bkqv = np.asarray(bkqv, np.float32)
    xt = np.ascontiguousarray(np.transpose(x, (0, 2, 1)))
    xt_b = xt.astype(bf16)
    xt_8 = xt.astype(fp8)
    pidx = np.arange(P)[:, None]
    fidx = np.arange(256)[None, :]
    masks = np.stack(
        [(pidx + P * i <= fidx) for i in range(2)]
    ).astype(bf16)
    ident = np.eye(P, dtype=bf16)
    Wk = Wkqv[:, :, :D]
    Wq = Wkqv[:, :, D:2 * D]
    in_maps = []
    for c in range(NCORES):
        m8 = np.empty((HL, D, D), fp8)
        t2 = np.empty((HL, B, P, NT), np.float32)
        for hh in range(HL):
            h = HL * c + hh
            m8[hh] = ((Wq[h] @ Wk[h].T) * SM).astype(fp8)
            bq = bkqv[h, D:2 * D]
            bk = bkqv[h, :D]
            t2v = (x @ (Wk[h] @ bq) + bq @ bk) / 32.0     # [B, N]
            t2[hh] = t2v.reshape(B, NT, P).transpose(0, 2, 1)
        wv = np.ascontiguousarray(
            np.concatenate(
                [Wkqv[HL * c + hh, :, 2 * D:] for hh in range(HL)], axis=1
            )
        ).astype(bf16)
        biasv = np.concatenate(
            [bkqv[HL * c + hh, 2 * D:] for hh in range(HL)]
        ).astype(np.float32)[:, None]
        wp = np.ascontiguousarray(Wp[P * c:P * (c + 1)]).astype(bf16)
        in_maps.append({
            "xt": xt_b, "xt8": xt_8, "m8": m8, "wv": wv, "biasv": biasv,
            "wp": wp, "masks": masks, "ident": ident, "t2": t2,
        })
    return in_maps


def run(x, Wkqv, bkqv, Wp, bp, trace=False):
    nc = _get_nc()
    in_maps = make_in_maps(x, Wkqv, bkqv, Wp)
    res = run_bass_kernel_spmd(nc, in_maps, core_ids=list(range(NCORES)), trace=trace)
    total = None
    for r in res.results:
        part = r["out"].astype(np.float64)
        total = part if total is None else total + part
    out = (total + np.asarray(bp, np.float64)).astype(np.float32)
    return out, res


def kernel(x, Wkqv, bkqv, Wp, bp):
    out, _ = run(x, Wkqv, bkqv, Wp, bp, trace=False)
    return out


# revision 20
# speedup vs baseline: 1.1978x; 1.0395x over previous
"""Causal self-attention (per-head full-D k/q, DH-wide v) on 8 trn2 cores.

Sharding: tensor-parallel over heads. Core c owns heads (2c, 2c+1).

Algebraic fusion: only S = q@k^T is needed (q, k are never output), so the
host precomputes M[h] = Wq[h] @ Wk[h]^T (a weight-only transform, 0.3s on
CPU) and the device computes

  z^T[h]  = M[h]-contraction @ x^T         (one projection instead of two)
  S^T     = x @ z^T                        (keys are raw x — no k-proj!)

which halves the dominant projection FLOPs vs the q/k form. The k/q biases
fold exactly into softmax: the bq-side term is constant per query and
cancels; the bk-side term2[m] = x[m]·(Wk bq) rides the exp as a
per-partition ACT bias (zeros for this problem's inputs, exact in general).

fp8: M and x ship as TRN fp8e4 (M scaled by 64 to center its range), z is
requantized to fp8e4, and the z-projection and S matmuls run as DoubleRow
fp8 (2 k-tiles per instruction, 157 TF/s). The v path, A@v, and output
projection stay bf16 (calibrated: fp8 there blows the 2e-2 budget; this
config measures rel_absmax ~1.4e-2).

Per core, per batch (phase-grouped by PE matmul mode to minimize
fp8<->bf16 transitions):
  [bf16] v^T packed projection (both heads in one 128-row group), deferred
         output projections of the PREVIOUS batch, v transposes
  per head h:
    [DR]   z^T(h): each weight pair feeds both 512-wide n blocks
    [DR]   S^T(h): one chain per key tile m over its full valid 256-aligned
           n range, chunked at 512 for PSUM (48 matmuls + 32 weight loads
           per head-batch instead of 80 + 80); exp with t2 ACT bias; the
           first 256 columns of each P^T tile are masked in place (they are
           only ever consumed as the diagonal block)
    [bf16] A@v chains per 256-wide n block + normalize
Host sums the 8 partial output projections and adds bp.

Scheduling details:
- P^T tiles per m-tile are exactly the moving operands the A@v chains want;
  the softmax denominator rides as a ones-column inside A@v (row 64).
- normalization: ACT copy -> GpSimd broadcast -> DVE approx-recip (~18
  bits, 5x cheaper than full reciprocal) / mul — never touches the PE.
- output projections run a full batch late (blocks 0 at the end of the A@v
  phase, blocks 1..3 in the next batch's bf16 window) so their operands are
  always long ready when the PE reaches them.
"""

import sys
import types

import numpy as np
import ml_dtypes

import concourse.bass as bass
import concourse.bacc as bacc
import concourse.tile as tile
from concourse import mybir
from concourse.bass_utils import run_bass_kernel_spmd

# If BASS_TRACE is set in the environment, run_bass_kernel_spmd imports
# antenv.axon_hooks, which this image may not ship. Register a stub that
# reports "no hook" so tracing degrades gracefully instead of crashing.
try:
    from antenv.axon_hooks import get_axon_ntff_profile_hook  # noqa: F401
except ImportError:
    import antenv

    _mod = types.ModuleType("antenv.axon_hooks")
    _mod.get_axon_ntff_profile_hook = lambda: None
    _mod.set_axon_ntff_profile_hook = lambda h: setattr(
        _mod, "get_axon_ntff_profile_hook", lambda: h
    )
    antenv.axon_hooks = _mod
    sys.modules["antenv.axon_hooks"] = _mod

BF16 = mybir.dt.bfloat16
F32 = mybir.dt.float32
FP8 = mybir.dt.float8e4
AF = mybir.ActivationFunctionType
DR = mybir.MatmulPerfMode.DoubleRow

B, N, D, H, DH = 4, 1024, 1024, 16, 64
P = 128
NCORES = 8
HL = H // NCORES        # 2 local heads per core
DT = D // P             # 8 contraction tiles
KP = DT // 2            # 4 DoubleRow k-pair steps
NB = N // 512           # 2 moving-dim blocks (projection / output)
NS = N // 256           # 4 moving-dim blocks (A@v)
NT = N // P             # 8 token tiles
SM = 64.0               # M pre-scale so fp8e4 sees a centered range
VG = 96                 # v slot stride: 64 v cols + ones col, 32-aligned

_CACHE = {}


def _build_nc():
    nc = bacc.Bacc(
        "TRN2",
        target_bir_lowering=False,
        debug=False,
        enable_asserts=True,
        num_devices=NCORES,
    )
    xt_d = nc.declare_dram_parameter("xt", [B, D, N], BF16, isOutput=False)
    xt8_d = nc.declare_dram_parameter("xt8", [B, D, N], FP8, isOutput=False)
    m8_d = nc.declare_dram_parameter("m8", [HL, D, D], FP8, isOutput=False)
    wv_d = nc.declare_dram_parameter("wv", [D, HL * DH], BF16, isOutput=False)
    biasv_d = nc.declare_dram_parameter("biasv", [P, 1], F32, isOutput=False)
    wp_d = nc.declare_dram_parameter("wp", [P, D], BF16, isOutput=False)
    mask_d = nc.declare_dram_parameter("masks", [2, P, 256], BF16, isOutput=False)
    id_d = nc.declare_dram_parameter("ident", [P, P], BF16, isOutput=False)
    t2_d = nc.declare_dram_parameter("t2", [HL, B, P, NT], F32, isOutput=False)
    out_d = nc.declare_dram_parameter("out", [B, N, D], F32, isOutput=True)

    with tile.TileContext(nc) as tc:
        with (
            tc.tile_pool(name="const", bufs=1) as constp,
            tc.tile_pool(name="mpool", bufs=1) as mpool,
            tc.tile_pool(name="xpool", bufs=2) as xpool,
            tc.tile_pool(name="zpool", bufs=2) as zpool,
            tc.tile_pool(name="vpool", bufs=2) as vpool,
            tc.tile_pool(name="ptpool", bufs=2) as ptpool,
            tc.tile_pool(name="otpool", bufs=2) as otpool,
            tc.tile_pool(name="stpool", bufs=3) as stpool,
            tc.tile_pool(name="pspool", bufs=1, space="PSUM") as pspool,
        ):
            m8_sb = mpool.tile([P, HL * DT, D], FP8, name="m8_sb")
            wv_sb = constp.tile([P, DT * HL * DH], BF16, name="wv_sb")
            biasv_sb = constp.tile([P, 1], F32, name="biasv_sb")
            wp_sb = constp.tile([P, D], BF16, name="wp_sb")
            mask_sb = constp.tile([P, 2 * 256], BF16, name="mask_sb")
            id_sb = constp.tile([P, P], BF16, name="id_sb")
            t2_sb = constp.tile([P, HL * B * NT], F32, name="t2_sb")

            def dma_m8(h, d):
                nc.sync.dma_start(
                    out=m8_sb[:, h * DT + d:h * DT + d + 1, :],
                    in_=m8_d[h, d * P:(d + 1) * P, :],
                )

            nc.sync.dma_start(  # first v-weight tile: the packed v matmuls open batch 0
                out=wv_sb[:, 0:P], in_=wv_d[0:P, :]
            )

            prev_final = None  # deferred output-projection emission

            def emit_final(ctx2):
                bb, ost, tiles = ctx2
                for t in tiles:
                    for j2 in range(NB):
                        ps_f = pspool.tile([P, 512], F32, tag="psf", bufs=2, name="ps_f")
                        nc.tensor.matmul(
                            ps_f[:],
                            lhsT=ost[:, t * P:(t + 1) * P],
                            rhs=wp_sb[:, j2 * 512:(j2 + 1) * 512],
                            start=True, stop=True,
                        )
                        stage = stpool.tile([P, 512], F32, tag="stage", name="stage")
                        if (t * NB + j2) % 2 == 0:
                            nc.scalar.activation(stage[:], ps_f[:], AF.Copy)
                        else:
                            nc.vector.tensor_copy(stage[:], ps_f[:])
                        nc.sync.dma_start(
                            out=out_d[bb, t * P:(t + 1) * P, j2 * 512:(j2 + 1) * 512],
                            in_=stage[:],
                        )

            for b in range(B):
                xt_sb = xpool.tile([P, DT * N], BF16, tag="xt", name="xt_sb")
                xt8_sb = xpool.tile([P, DT, N], FP8, tag="xt8", name="xt8_sb")
                for d in range(DT):
                    if b == 0 and d == 0:
                        nc.sync.dma_start(out=biasv_sb[:], in_=biasv_d[:])
                    nc.sync.dma_start(
                        out=xt_sb[:, d * N:(d + 1) * N],
                        in_=xt_d[b, d * P:(d + 1) * P, :],
                    )
                    if b == 0 and d >= 1:
                        nc.sync.dma_start(
                            out=wv_sb[:, d * P:(d + 1) * P],
                            in_=wv_d[d * P:(d + 1) * P, :],
                        )
                    nc.sync.dma_start(
                        out=xt8_sb[:, d:d + 1, :],
                        in_=xt8_d[b, d * P:(d + 1) * P, :],
                    )
                    if b == 0:
                        dma_m8(0, d)
                if b == 0:
                    nc.sync.dma_start(out=id_sb[:], in_=id_d[:])
                    for m in range(2):
                        nc.sync.dma_start(
                            out=mask_sb[:, m * 256:(m + 1) * 256], in_=mask_d[m]
                        )
                    for d in range(DT):
                        dma_m8(1, d)
                    nc.sync.dma_start(out=wp_sb[:], in_=wp_d[:])
                    for h in range(HL):
                        for bb in range(B):
                            nc.sync.dma_start(
                                out=t2_sb[:, (h * B + bb) * NT:(h * B + bb + 1) * NT],
                                in_=t2_d[h, bb],
                            )
                ostack = otpool.tile([P, N], BF16, tag="ostack", name="ostack")

                for h in range(HL):
                    if h == 0:
                        # v computed DIRECTLY in [tokens, dh] layout — the
                        # layout the A@v chains consume — so the PE never runs
                        # a transpose (no transpose-mode toggles): per token
                        # tile, lhsT = x^T slice (tokens -> out partitions),
                        # rhs = packed Wv (both heads' 64 columns). The v bias
                        # moves to an exact post-normalize add (softmax rows
                        # sum to 1, so A@(v+bv) = A@v + bv).
                        v_sb = vpool.tile(
                            [P, HL * NT * VG], BF16, tag="vaug", name="v_sb"
                        )
                        if prev_final is not None:
                            emit_final(prev_final)
                            prev_final = None
                        nc.vector.memset(v_sb[:, :], 1.0)
                        for i in range(NT):
                            ps_v = pspool.tile([P, P], F32, tag="ps", bufs=4, name="ps_v")
                            for d in range(DT):
                                nc.tensor.matmul(
                                    ps_v[:],
                                    lhsT=xt_sb[:, d * N + i * P: d * N + i * P + P],
                                    rhs=wv_sb[:, d * P:(d + 1) * P],
                                    start=(d == 0),
                                    stop=(d == DT - 1),
                                )
                            for hh in range(HL):
                                o0 = (hh * NT + i) * VG
                                nc.scalar.activation(
                                    v_sb[:, o0:o0 + DH],
                                    ps_v[:, hh * DH:(hh + 1) * DH],
                                    AF.Copy,
                                )

                    # ---- DR phase: z projection; each weight pair feeds
                    # both 512-wide n blocks back-to-back ----
                    zt8 = zpool.tile([P, DT, N], FP8, tag="zt", name="zt8")
                    for e in range(DT):
                        ps_za = pspool.tile([P, 512], F32, tag="ps", bufs=4, name="ps_za")
                        ps_zb = pspool.tile([P, 512], F32, tag="ps", bufs=4, name="ps_zb")
                        for d in range(KP):
                            for ps_z, j in ((ps_za, 0), (ps_zb, 1)):
                                nc.tensor.matmul(
                                    ps_z[:],
                                    lhsT=m8_sb[:, h * DT + 2 * d:h * DT + 2 * d + 2, e * P:(e + 1) * P],
                                    rhs=xt8_sb[:, 2 * d:2 * d + 2, j * 512:(j + 1) * 512],
                                    start=(d == 0),
                                    stop=(d == KP - 1),
                                    perf_mode=DR,
                                )
                        nc.scalar.activation(zt8[:, e:e + 1, 0:512], ps_za[:], AF.Copy)
                        nc.vector.tensor_copy(zt8[:, e:e + 1, 512:N], ps_zb[:])

                    # ---- DR phase: S^T, one chain per key tile m over its
                    # full valid 256-aligned n range, chunked at 512 ----
                    pts = []
                    for i in range(NT):
                        jb = i // 2
                        w = N - 256 * jb
                        chunks = [(s, min(512, w - s)) for s in range(0, w, 512)]
                        pss = [
                            pspool.tile([P, cw], F32, tag="ps", bufs=4, name="ps_s")
                            for (s, cw) in chunks
                        ]
                        for d in range(KP):
                            for ci, (s, cw) in enumerate(chunks):
                                nc.tensor.matmul(
                                    pss[ci][:],
                                    lhsT=xt8_sb[:, 2 * d:2 * d + 2, i * P:(i + 1) * P],
                                    rhs=zt8[:, 2 * d:2 * d + 2, 256 * jb + s:256 * jb + s + cw],
                                    start=(d == 0),
                                    stop=(d == KP - 1),
                                    perf_mode=DR,
                                )
                        pt = ptpool.tile([P, w], BF16, tag=f"pt{i}", bufs=2, name=f"pt{i}")
                        t2i = (h * B + b) * NT + i
                        for ci, (s, cw) in enumerate(chunks):
                            nc.scalar.activation(
                                pt[:, s:s + cw], pss[ci][:], AF.Exp,
                                scale=1.0 / (32.0 * SM),
                                bias=t2_sb[:, t2i:t2i + 1],
                            )
                        # the first 256 columns are only ever consumed as the
                        # masked diagonal block — mask them in place
                        nc.vector.tensor_mul(
                            pt[:, 0:256], pt[:, 0:256],
                            mask_sb[:, (i % 2) * 256:(i % 2 + 1) * 256],
                        )
                        pts.append((pt, jb))

                    # ---- bf16 phase: A@v chains per 256-wide n block;
                    # diagonal blocks read the in-place-masked first 256
                    # columns of their pt tile ----
                    for j in range(NS):
                        if h == HL - 1 and j == NS - 1:
                            emit_final((b, ostack, (0, 1) if b < B - 1 else (0, 1, 2, 3)))
                        ps_o = pspool.tile([DH + 1, 256], F32, tag="po", bufs=2, name="ps_o")
                        for i in range(2 * j + 2):
                            pt, jb = pts[i]
                            rhs = pt[:, (j - jb) * 256:(j - jb + 1) * 256]
                            nc.tensor.matmul(
                                ps_o[:],
                                lhsT=v_sb[:, (h * NT + i) * VG:(h * NT + i) * VG + DH + 1],
                                rhs=rhs,
                                start=(i == 0),
                                stop=(i == 2 * j + 1),
                            )
                        den_row = otpool.tile([1, 256], F32, tag="den", name="den_row")
                        nc.scalar.activation(den_row[:], ps_o[DH:DH + 1, :], AF.Copy)
                        ot = otpool.tile([DH, 256], F32, tag="ot", name="ot")
                        nc.scalar.activation(ot[:], ps_o[:DH, :], AF.Copy)
                        den_b = stpool.tile([DH, 256], F32, tag="denb", name="den_b")
                        nc.gpsimd.partition_broadcast(den_b[:], den_row[:], channels=DH)
                        recip = stpool.tile([DH, 256], F32, tag="recip", name="recip")
                        nc.vector.reciprocal_approx_fast(recip[:], den_b[:])
                        onorm = stpool.tile([DH, 256], F32, tag="onrm", name="onorm")
                        nc.vector.tensor_mul(onorm[:], ot[:], recip[:])
                        # exact v-bias: softmax rows sum to 1 -> + bv per head dim
                        nc.vector.tensor_scalar_add(
                            ostack[h * DH:(h + 1) * DH, j * 256:(j + 1) * 256],
                            onorm[:], biasv_sb[h * DH:(h + 1) * DH, 0:1],
                        )
                # output projection: blocks 1..3 deferred into the next
                # batch's bf16 window (last batch: emitted right here)
                if b < B - 1:
                    prev_final = (b, ostack, (2, 3, 4, 5, 6, 7))
                else:
                    emit_final((b, ostack, (4, 5, 6, 7)))
    nc.finalize()
    return nc


def _get_nc():
    if "nc" not in _CACHE:
        _CACHE["nc"] = _build_nc()
    return _CACHE["nc"]


def make_in_maps(x, Wkqv, bkqv, Wp):
    bf16 = ml_dtypes.bfloat16
    fp8 = ml_dtypes.float8_e4m3
    x = np.asarray(x, np.float32)
    Wkqv = np.asarray(Wkqv, np.float32)
    bkqv = np.asarray(bkqv, np.float32)
    xt = np.ascontiguousarray(np.transpose(x, (0, 2, 1)))
    xt_b = xt.astype(bf16)
    xt_8 = xt.astype(fp8)
    pidx = np.arange(P)[:, None]
    fidx = np.arange(256)[None, :]
    masks = np.stack(
        [(pidx + P * i <= fidx) for i in range(2)]
    ).astype(bf16)
    ident = np.eye(P, dtype=bf16)
    Wk = Wkqv[:, :, :D]
    Wq = Wkqv[:, :, D:2 * D]
    in_maps = []
    for c in range(NCORES):
        m8 = np.empty((HL, D, D), fp8)
        t2 = np.empty((HL, B, P, NT), np.float32)
        for hh in range(HL):
            h = HL * c + hh
            m8[hh] = ((Wq[h] @ Wk[h].T) * SM).astype(fp8)
            bq = bkqv[h, D:2 * D]
            bk = bkqv[h, :D]
            t2v = (x @ (Wk[h] @ bq) + bq @ bk) / 32.0     # [B, N]
            t2[hh] = t2v.reshape(B, NT, P).transpose(0, 2, 1)
        wv = np.ascontiguousarray(
            np.concatenate(
                [Wkqv[HL * c + hh, :, 2 * D:] for hh in range(HL)], axis=1
            )
        ).astype(bf16)
        biasv = np.concatenate(
            [bkqv[HL * c + hh, 2 * D:] for hh in range(HL)]
        ).astype(np.float32)[:, None]
        wp = np.ascontiguousarray(Wp[P * c:P * (c + 1)]).astype(bf16)
        in_maps.append({
            "xt": xt_b, "xt8": xt_8, "m8": m8, "wv": wv, "biasv": biasv,
            "wp": wp, "masks": masks, "ident": ident, "t2": t2,
        })
    return in_maps


def run(x, Wkqv, bkqv, Wp, bp, trace=False):
    nc = _get_nc()
    in_maps = make_in_maps(x, Wkqv, bkqv, Wp)
    res = run_bass_kernel_spmd(nc, in_maps, core_ids=list(range(NCORES)), trace=trace)
    total = None
    for r in res.results:
        part = r["out"].astype(np.float64)
        total = part if total is None else total + part
    out = (total + np.asarray(bp, np.float64)).astype(np.float32)
    return out, res


def kernel(x, Wkqv, bkqv, Wp, bp):
    out, _ = run(x, Wkqv, bkqv, Wp, bp, trace=False)
    return out


# revision 21
# speedup vs baseline: 1.2117x; 1.0116x over previous
"""Causal self-attention (per-head full-D k/q, DH-wide v) on 8 trn2 cores.

Sharding: tensor-parallel over heads. Core c owns heads (2c, 2c+1).

Algebraic fusion: only S = q@k^T is needed (q, k are never output), so the
host precomputes M[h] = Wq[h] @ Wk[h]^T (a weight-only transform, 0.3s on
CPU) and the device computes

  z^T[h]  = M[h]-contraction @ x^T         (one projection instead of two)
  S^T     = x @ z^T                        (keys are raw x — no k-proj!)

which halves the dominant projection FLOPs vs the q/k form. The k/q biases
fold exactly into softmax: the bq-side term is constant per query and
cancels; the bk-side term2[m] = x[m]·(Wk bq) rides the exp as a
per-partition ACT bias (zeros for this problem's inputs, exact in general).

fp8: M and x ship as TRN fp8e4 (M scaled by 64 to center its range), z is
requantized to fp8e4, and the z-projection and S matmuls run as DoubleRow
fp8 (2 k-tiles per instruction, 157 TF/s). The v path, A@v, and output
projection stay bf16 (calibrated: fp8 there blows the 2e-2 budget; this
config measures rel_absmax ~1.4e-2).

Per core, per batch (phase-grouped by PE matmul mode to minimize
fp8<->bf16 transitions):
  [bf16] v^T packed projection (both heads in one 128-row group), deferred
         output projections of the PREVIOUS batch, v transposes
  per head h:
    [DR]   z^T(h): each weight pair feeds both 512-wide n blocks
    [DR]   S^T(h): one chain per key tile m over its full valid 256-aligned
           n range, chunked at 512 for PSUM (48 matmuls + 32 weight loads
           per head-batch instead of 80 + 80); exp with t2 ACT bias; the
           first 256 columns of each P^T tile are masked in place (they are
           only ever consumed as the diagonal block)
    [bf16] A@v chains per 256-wide n block + normalize
Host sums the 8 partial output projections and adds bp.

Scheduling details:
- P^T tiles per m-tile are exactly the moving operands the A@v chains want;
  the softmax denominator rides as a ones-column inside A@v (row 64).
- normalization: ACT copy -> GpSimd broadcast -> DVE approx-recip (~18
  bits, 5x cheaper than full reciprocal) / mul — never touches the PE.
- output projections run a full batch late (blocks 0 at the end of the A@v
  phase, blocks 1..3 in the next batch's bf16 window) so their operands are
  always long ready when the PE reaches them.
"""

import sys
import types

import numpy as np
import ml_dtypes

import concourse.bass as bass
import concourse.bacc as bacc
import concourse.tile as tile
from concourse import mybir
from concourse.bass_utils import run_bass_kernel_spmd

# If BASS_TRACE is set in the environment, run_bass_kernel_spmd imports
# antenv.axon_hooks, which this image may not ship. Register a stub that
# reports "no hook" so tracing degrades gracefully instead of crashing.
try:
    from antenv.axon_hooks import get_axon_ntff_profile_hook  # noqa: F401
except ImportError:
    import antenv

    _mod = types.ModuleType("antenv.axon_hooks")
    _mod.get_axon_ntff_profile_hook = lambda: None
    _mod.set_axon_ntff_profile_hook = lambda h: setattr(
        _mod, "get_axon_ntff_profile_hook", lambda: h
    )
    antenv.axon_hooks = _mod
    sys.modules["antenv.axon_hooks"] = _mod

BF16 = mybir.dt.bfloat16
F32 = mybir.dt.float32
FP8 = mybir.dt.float8e4
AF = mybir.ActivationFunctionType
DR = mybir.MatmulPerfMode.DoubleRow

B, N, D, H, DH = 4, 1024, 1024, 16, 64
P = 128
NCORES = 8
HL = H // NCORES        # 2 local heads per core
DT = D // P             # 8 contraction tiles
KP = DT // 2            # 4 DoubleRow k-pair steps
NB = N // 512           # 2 moving-dim blocks (projection / output)
NS = N // 256           # 4 moving-dim blocks (A@v)
NT = N // P             # 8 token tiles
SM = 64.0               # M pre-scale so fp8e4 sees a centered range
VG = 96                 # v slot stride: 64 v cols + ones col, 32-aligned

_CACHE = {}


def _build_nc():
    nc = bacc.Bacc(
        "TRN2",
        target_bir_lowering=False,
        debug=False,
        enable_asserts=True,
        num_devices=NCORES,
    )
    xt_d = nc.declare_dram_parameter("xt", [B, D, N], BF16, isOutput=False)
    xt8_d = nc.declare_dram_parameter("xt8", [B, D, N], FP8, isOutput=False)
    m8_d = nc.declare_dram_parameter("m8", [HL, D, D], FP8, isOutput=False)
    wv_d = nc.declare_dram_parameter("wv", [D, HL * DH], BF16, isOutput=False)
    biasv_d = nc.declare_dram_parameter("biasv", [P, 1], F32, isOutput=False)
    wp_d = nc.declare_dram_parameter("wp", [P, D], BF16, isOutput=False)
    mask_d = nc.declare_dram_parameter("masks", [2, P, 256], BF16, isOutput=False)
    id_d = nc.declare_dram_parameter("ident", [P, P], BF16, isOutput=False)
    t2_d = nc.declare_dram_parameter("t2", [HL, B, P, NT], F32, isOutput=False)
    out_d = nc.declare_dram_parameter("out", [B, N, D], F32, isOutput=True)

    with tile.TileContext(nc) as tc:
        with (
            tc.tile_pool(name="const", bufs=1) as constp,
            tc.tile_pool(name="mpool", bufs=1) as mpool,
            tc.tile_pool(name="xpool", bufs=2) as xpool,
            tc.tile_pool(name="zpool", bufs=2) as zpool,
            tc.tile_pool(name="vpool", bufs=2) as vpool,
            tc.tile_pool(name="ptpool", bufs=2) as ptpool,
            tc.tile_pool(name="otpool", bufs=2) as otpool,
            tc.tile_pool(name="stpool", bufs=3) as stpool,
            tc.tile_pool(name="pspool", bufs=1, space="PSUM") as pspool,
        ):
            m8_sb = mpool.tile([P, HL * DT, D], FP8, name="m8_sb")
            wv_sb = constp.tile([P, DT * HL * DH], BF16, name="wv_sb")
            biasv_sb = constp.tile([P, 1], F32, name="biasv_sb")
            wp_sb = constp.tile([P, D], BF16, name="wp_sb")
            mask_sb = constp.tile([P, 2 * 256], BF16, name="mask_sb")
            id_sb = constp.tile([P, P], BF16, name="id_sb")
            t2_sb = constp.tile([P, HL * B * NT], F32, name="t2_sb")

            def dma_m8(h, d):
                nc.sync.dma_start(
                    out=m8_sb[:, h * DT + d:h * DT + d + 1, :],
                    in_=m8_d[h, d * P:(d + 1) * P, :],
                )

            nc.sync.dma_start(  # first v-weight tile: the packed v matmuls open batch 0
                out=wv_sb[:, 0:P], in_=wv_d[0:P, :]
            )

            prev_final = None  # deferred output-projection emission

            def emit_final(ctx2):
                bb, ost, tiles = ctx2
                for t in tiles:
                    for j2 in range(NB):
                        ps_f = pspool.tile([P, 512], F32, tag="psf", bufs=2, name="ps_f")
                        nc.tensor.matmul(
                            ps_f[:],
                            lhsT=ost[:, t * P:(t + 1) * P],
                            rhs=wp_sb[:, j2 * 512:(j2 + 1) * 512],
                            start=True, stop=True,
                        )
                        stage = stpool.tile([P, 512], F32, tag="stage", name="stage")
                        if (t * NB + j2) % 2 == 0:
                            nc.scalar.activation(stage[:], ps_f[:], AF.Copy)
                        else:
                            nc.vector.tensor_copy(stage[:], ps_f[:])
                        nc.sync.dma_start(
                            out=out_d[bb, t * P:(t + 1) * P, j2 * 512:(j2 + 1) * 512],
                            in_=stage[:],
                        )

            for b in range(B):
                xt_sb = xpool.tile([P, DT * N], BF16, tag="xt", name="xt_sb")
                xt8_sb = xpool.tile([P, DT, N], FP8, tag="xt8", name="xt8_sb")
                # batch 0: v-proj operands (xt, wv) first so the opening
                # bf16 window starts as early as possible; the fp8 operands
                # land during it, in time for the first DR phase
                for d in range(DT):
                    if b == 0 and d == 0:
                        nc.sync.dma_start(out=biasv_sb[:], in_=biasv_d[:])
                    nc.sync.dma_start(
                        out=xt_sb[:, d * N:(d + 1) * N],
                        in_=xt_d[b, d * P:(d + 1) * P, :],
                    )
                    if b == 0 and d >= 1:
                        nc.sync.dma_start(
                            out=wv_sb[:, d * P:(d + 1) * P],
                            in_=wv_d[d * P:(d + 1) * P, :],
                        )
                    if b > 0:
                        nc.sync.dma_start(
                            out=xt8_sb[:, d:d + 1, :],
                            in_=xt8_d[b, d * P:(d + 1) * P, :],
                        )
                for d in range(DT):
                    if b == 0:
                        nc.sync.dma_start(
                            out=xt8_sb[:, d:d + 1, :],
                            in_=xt8_d[b, d * P:(d + 1) * P, :],
                        )
                        dma_m8(0, d)
                if b == 0:
                    nc.sync.dma_start(out=id_sb[:], in_=id_d[:])
                    for m in range(2):
                        nc.sync.dma_start(
                            out=mask_sb[:, m * 256:(m + 1) * 256], in_=mask_d[m]
                        )
                    for d in range(DT):
                        dma_m8(1, d)
                    nc.sync.dma_start(out=wp_sb[:], in_=wp_d[:])
                    for h in range(HL):
                        for bb in range(B):
                            nc.sync.dma_start(
                                out=t2_sb[:, (h * B + bb) * NT:(h * B + bb + 1) * NT],
                                in_=t2_d[h, bb],
                            )
                ostack = otpool.tile([P, N], BF16, tag="ostack", name="ostack")

                for h in range(HL):
                    if h == 0:
                        # v computed DIRECTLY in [tokens, dh] layout — the
                        # layout the A@v chains consume — so the PE never runs
                        # a transpose (no transpose-mode toggles): per token
                        # tile, lhsT = x^T slice (tokens -> out partitions),
                        # rhs = packed Wv (both heads' 64 columns). The v bias
                        # moves to an exact post-normalize add (softmax rows
                        # sum to 1, so A@(v+bv) = A@v + bv).
                        v_sb = vpool.tile(
                            [P, HL * NT * VG], BF16, tag="vaug", name="v_sb"
                        )
                        if prev_final is not None:
                            emit_final(prev_final)
                            prev_final = None
                        nc.vector.memset(v_sb[:, :], 1.0)
                        for i in range(NT):
                            ps_v = pspool.tile([P, P], F32, tag="ps", bufs=4, name="ps_v")
                            for d in range(DT):
                                nc.tensor.matmul(
                                    ps_v[:],
                                    lhsT=xt_sb[:, d * N + i * P: d * N + i * P + P],
                                    rhs=wv_sb[:, d * P:(d + 1) * P],
                                    start=(d == 0),
                                    stop=(d == DT - 1),
                                )
                            for hh in range(HL):
                                o0 = (hh * NT + i) * VG
                                nc.scalar.activation(
                                    v_sb[:, o0:o0 + DH],
                                    ps_v[:, hh * DH:(hh + 1) * DH],
                                    AF.Copy,
                                )

                    # ---- DR phase: z projection; each weight pair feeds
                    # both 512-wide n blocks back-to-back ----
                    zt8 = zpool.tile([P, DT, N], FP8, tag="zt", name="zt8")
                    for e in range(DT):
                        ps_za = pspool.tile([P, 512], F32, tag="ps", bufs=4, name="ps_za")
                        ps_zb = pspool.tile([P, 512], F32, tag="ps", bufs=4, name="ps_zb")
                        for d in range(KP):
                            for ps_z, j in ((ps_za, 0), (ps_zb, 1)):
                                nc.tensor.matmul(
                                    ps_z[:],
                                    lhsT=m8_sb[:, h * DT + 2 * d:h * DT + 2 * d + 2, e * P:(e + 1) * P],
                                    rhs=xt8_sb[:, 2 * d:2 * d + 2, j * 512:(j + 1) * 512],
                                    start=(d == 0),
                                    stop=(d == KP - 1),
                                    perf_mode=DR,
                                )
                        nc.scalar.activation(zt8[:, e:e + 1, 0:512], ps_za[:], AF.Copy)
                        nc.vector.tensor_copy(zt8[:, e:e + 1, 512:N], ps_zb[:])

                    # ---- DR phase: S^T, one chain per key tile m over its
                    # full valid 256-aligned n range, chunked at 512 ----
                    pts = []
                    for i in range(NT):
                        jb = i // 2
                        w = N - 256 * jb
                        chunks = [(s, min(512, w - s)) for s in range(0, w, 512)]
                        pss = [
                            pspool.tile([P, cw], F32, tag="ps", bufs=4, name="ps_s")
                            for (s, cw) in chunks
                        ]
                        for d in range(KP):
                            for ci, (s, cw) in enumerate(chunks):
                                nc.tensor.matmul(
                                    pss[ci][:],
                                    lhsT=xt8_sb[:, 2 * d:2 * d + 2, i * P:(i + 1) * P],
                                    rhs=zt8[:, 2 * d:2 * d + 2, 256 * jb + s:256 * jb + s + cw],
                                    start=(d == 0),
                                    stop=(d == KP - 1),
                                    perf_mode=DR,
                                )
                        pt = ptpool.tile([P, w], BF16, tag=f"pt{i}", bufs=2, name=f"pt{i}")
                        t2i = (h * B + b) * NT + i
                        for ci, (s, cw) in enumerate(chunks):
                            nc.scalar.activation(
                                pt[:, s:s + cw], pss[ci][:], AF.Exp,
                                scale=1.0 / (32.0 * SM),
                                bias=t2_sb[:, t2i:t2i + 1],
                            )
                        # the first 256 columns are only ever consumed as the
                        # masked diagonal block — mask them in place
                        nc.vector.tensor_mul(
                            pt[:, 0:256], pt[:, 0:256],
                            mask_sb[:, (i % 2) * 256:(i % 2 + 1) * 256],
                        )
                        pts.append((pt, jb))

                    # ---- bf16 phase: A@v chains per 256-wide n block;
                    # diagonal blocks read the in-place-masked first 256
                    # columns of their pt tile ----
                    for j in range(NS):
                        if h == HL - 1 and j == NS - 1:
                            emit_final((b, ostack, (0, 1) if b < B - 1 else (0, 1, 2, 3)))
                        ps_o = pspool.tile([DH + 1, 256], F32, tag="po", bufs=2, name="ps_o")
                        for i in range(2 * j + 2):
                            pt, jb = pts[i]
                            rhs = pt[:, (j - jb) * 256:(j - jb + 1) * 256]
                            nc.tensor.matmul(
                                ps_o[:],
                                lhsT=v_sb[:, (h * NT + i) * VG:(h * NT + i) * VG + DH + 1],
                                rhs=rhs,
                                start=(i == 0),
                                stop=(i == 2 * j + 1),
                            )
                        den_row = otpool.tile([1, 256], F32, tag="den", name="den_row")
                        nc.scalar.activation(den_row[:], ps_o[DH:DH + 1, :], AF.Copy)
                        ot = otpool.tile([DH, 256], F32, tag="ot", name="ot")
                        nc.scalar.activation(ot[:], ps_o[:DH, :], AF.Copy)
                        den_b = stpool.tile([DH, 256], F32, tag="denb", name="den_b")
                        nc.gpsimd.partition_broadcast(den_b[:], den_row[:], channels=DH)
                        recip = stpool.tile([DH, 256], F32, tag="recip", name="recip")
                        nc.vector.reciprocal_approx_fast(recip[:], den_b[:])
                        onorm = stpool.tile([DH, 256], F32, tag="onrm", name="onorm")
                        nc.vector.tensor_mul(onorm[:], ot[:], recip[:])
                        # exact v-bias: softmax rows sum to 1 -> + bv per head dim
                        nc.vector.tensor_scalar_add(
                            ostack[h * DH:(h + 1) * DH, j * 256:(j + 1) * 256],
                            onorm[:], biasv_sb[h * DH:(h + 1) * DH, 0:1],
                        )
                # output projection: blocks 1..3 deferred into the next
                # batch's bf16 window (last batch: emitted right here)
                if b < B - 1:
                    prev_final = (b, ostack, (2, 3, 4, 5, 6, 7))
                else:
                    emit_final((b, ostack, (4, 5, 6, 7)))
    nc.finalize()
    return nc


def _get_nc():
    if "nc" not in _CACHE:
        _CACHE["nc"] = _build_nc()
    return _CACHE["nc"]


def make_in_maps(x, Wkqv, bkqv, Wp):
    bf16 = ml_dtypes.bfloat16
    fp8 = ml_dtypes.float8_e4m3
    x = np.asarray(x, np.float32)
    Wkqv = np.asarray(Wkqv, np.float32)
    bkqv = np.asarray(bkqv, np.float32)
    xt = np.ascontiguousarray(np.transpose(x, (0, 2, 1)))
    xt_b = xt.astype(bf16)
    xt_8 = xt.astype(fp8)
    pidx = np.arange(P)[:, None]
    fidx = np.arange(256)[None, :]
    masks = np.stack(
        [(pidx + P * i <= fidx) for i in range(2)]
    ).astype(bf16)
    ident = np.eye(P, dtype=bf16)
    Wk = Wkqv[:, :, :D]
    Wq = Wkqv[:, :, D:2 * D]
    in_maps = []
    for c in range(NCORES):
        m8 = np.empty((HL, D, D), fp8)
        t2 = np.empty((HL, B, P, NT), np.float32)
        for hh in range(HL):
            h = HL * c + hh
            m8[hh] = ((Wq[h] @ Wk[h].T) * SM).astype(fp8)
            bq = bkqv[h, D:2 * D]
            bk = bkqv[h, :D]
            t2v = (x @ (Wk[h] @ bq) + bq @ bk) / 32.0     # [B, N]
            t2[hh] = t2v.reshape(B, NT, P).transpose(0, 2, 1)
        wv = np.ascontiguousarray(
            np.concatenate(
                [Wkqv[HL * c + hh, :, 2 * D:] for hh in range(HL)], axis=1
            )
        ).astype(bf16)
        biasv = np.concatenate(
            [bkqv[HL * c + hh, 2 * D:] for hh in range(HL)]
        ).astype(np.float32)[:, None]
        wp = np.ascontiguousarray(Wp[P * c:P * (c + 1)]).astype(bf16)
        in_maps.append({
            "xt": xt_b, "xt8": xt_8, "m8": m8, "wv": wv, "biasv": biasv,
            "wp": wp, "masks": masks, "ident": ident, "t2": t2,
        })
    return in_maps


def run(x, Wkqv, bkqv, Wp, bp, trace=False):
    nc = _get_nc()
    in_maps = make_in_maps(x, Wkqv, bkqv, Wp)
    res = run_bass_kernel_spmd(nc, in_maps, core_ids=list(range(NCORES)), trace=trace)
    total = None
    for r in res.results:
        part = r["out"].astype(np.float64)
        total = part if total is None else total + part
    out = (total + np.asarray(bp, np.float64)).astype(np.float32)
    return out, res


def kernel(x, Wkqv, bkqv, Wp, bp):
    out, _ = run(x, Wkqv, bkqv, Wp, bp, trace=False)
    return out
